# revision 8
# baseline (speedup 1.0000x reference)
"""Single-head causal self-attention (B=4, S=4096, D=512) on 8 trn2 NeuronCores.

Sharding: 2 cores per batch element. Each core handles ALL queries of its
batch but only the even- or odd-indexed 128-row KV tiles (flash-decoding
style KV-parity split). This keeps the SPMD program identical across cores,
perfectly load-balances the causal triangle, and wastes no tiles.

Weight folding (host-side, f32): scores = x Wq^T Wk x^T = x G x^T with
G = Wq^T @ Wk, and the output projection is folded into the values:
Vt = V Wo^T = x (Wv^T Wo^T) = x H. The device then only computes
  R^T = (x G)^T           lhsT=G chunks, rhs=x^T          [b, q]
  Vt[k,e]                 lhsT=x^T local chunks, rhs=H    [k, e]
  S^T[k,q] = sum_b x^T[b,k]^T R[b,q]   (lhsT = x^T local) accum over b
  P^T      = exp(S^T * scale)          (ScalarE, PSUM->SBUF bf16)
  l[1,q]   = ones^T @ P^T              M=1 matmul, accum over k tiles in PSUM
  poT[e,q] = sum_k Vt[k,e]^T P^T[k,q]  accum over k tiles  (= final proj!)
This removes the K projection and the entire output projection from the PE,
and keeps the row-sum l off the slow engines entirely (PSUM accumulates).

Each core emits unnormalized partial outputs poT (already in output space)
and partial row-sums l. Host combines: out = ((poT0 + poT1) / (l0 + l1)).T
+ bo. Scores/sqrt(D) are bounded (~N(0, 0.33)) for well-scaled inputs, so
exp without max-subtraction is safe; softmax is shift-invariant so the
result matches the reference.
"""

import numpy as np
import ml_dtypes

import concourse.bass as bass
import concourse.tile as tile
from concourse import bacc, mybir
from concourse import bass_utils

B, S, D = 4, 4096, 512
TS = 128              # kv tile rows
QB = 512              # query block
NQB = S // QB         # 8 query blocks
NLT = S // TS // 2    # 16 local kv tiles per core
NDC = D // 128        # 4 chunks of d
SL = S // 2           # 2048 local kv rows
SCALE = 1.0 / float(np.sqrt(D))
BF16 = mybir.dt.bfloat16
F32 = mybir.dt.float32
FP8 = mybir.dt.float8e4
N_CORES = 8
USE_FP8_SCORES = True  # QK^T in fp8e4 DoubleRow (2 matmuls of K=256 per tile)


def build_kernel(repeat=1):
    nc = bacc.Bacc("TRN2", target_bir_lowering=False, debug=False)

    xfl = nc.dram_tensor("xfl", [D, SL], BF16, kind="ExternalInput").ap()
    if USE_FP8_SCORES:
        xfl8 = nc.dram_tensor("xfl8", [TS, NDC * SL], FP8, kind="ExternalInput").ap()
        xT8 = nc.dram_tensor("xT8", [TS, NDC * S], FP8, kind="ExternalInput").ap()
        g8 = nc.dram_tensor("g8", [TS, NDC * D], FP8, kind="ExternalInput").ap()
    else:
        xT = nc.dram_tensor("xT", [D, S], BF16, kind="ExternalInput").ap()
        g = nc.dram_tensor("g", [D, D], BF16, kind="ExternalInput").ap()
    h = nc.dram_tensor("h", [D, D], BF16, kind="ExternalInput").ap()
    masks = nc.dram_tensor("masks", [2 * TS, QB], BF16, kind="ExternalInput").ap()
    poT = nc.dram_tensor("poT", [D, S], BF16, kind="ExternalOutput").ap()
    lrow = nc.dram_tensor("lrow", [1, S], F32, kind="ExternalOutput").ap()

    with tile.TileContext(nc) as tc:
        with tc.tile_pool(name="persist", bufs=1) as P:
            xfl_sb = [P.tile([TS, SL], BF16, tag=f"xfl{e}", name=f"xfl{e}") for e in range(NDC)]
            h_sb = [P.tile([TS, D], BF16, tag=f"h{e}", name=f"h{e}") for e in range(NDC)]
            # DMA issue order = first-consumption order: Vt-proj needs xfl+h
            # first, then R-proj needs xT8+g8. x is split into 512-col pieces
            # so the first matmuls can start as soon as the leading columns of
            # every e-chunk land.
            for cb in range(SL // QB):
                for e in range(NDC):
                    nc.sync.dma_start(xfl_sb[e][:, cb * QB:(cb + 1) * QB],
                                      xfl[e * TS:(e + 1) * TS, cb * QB:(cb + 1) * QB])
                if cb == 0:
                    for e in range(NDC):
                        nc.sync.dma_start(h_sb[e][:], h[e * TS:(e + 1) * TS, :])
            if USE_FP8_SCORES:
                g8_sb = P.tile([TS, NDC, D], FP8, tag="g8", name="g8")
                for c in range(NDC):
                    nc.sync.dma_start(g8_sb[:, c, :], g8[:, c * D:(c + 1) * D])
                xT8_sb = P.tile([TS, NDC, S], FP8, tag="xT8", name="xT8")
                for cb in range(S // QB):
                    for c in range(NDC):
                        nc.sync.dma_start(
                            xT8_sb[:, c, cb * QB:(cb + 1) * QB],
                            xT8[:, c * S + cb * QB:c * S + (cb + 1) * QB])
            else:
                xT_sb = [P.tile([TS, S], BF16, tag=f"xT{e}", name=f"xT{e}") for e in range(NDC)]
                g_sb = [P.tile([TS, D], BF16, tag=f"g{e}", name=f"g{e}") for e in range(NDC)]
                for e in range(NDC):
                    nc.sync.dma_start(g_sb[e][:], g[e * TS:(e + 1) * TS, :])
                for cb in range(S // QB):
                    for e in range(NDC):
                        nc.sync.dma_start(xT_sb[e][:, cb * QB:(cb + 1) * QB],
                                          xT[e * TS:(e + 1) * TS, cb * QB:(cb + 1) * QB])
            mask_sb = [P.tile([TS, QB], BF16, tag=f"mask{c}", name=f"mask{c}") for c in range(2)]
            for c in range(2):
                nc.sync.dma_start(mask_sb[c][:], masks[c * TS:(c + 1) * TS, :])
            if USE_FP8_SCORES:
                # x^T local in fp8, d-chunks stacked along free: [128, chunk, col]
                xfl8_sb = P.tile([TS, NDC, SL], FP8, tag="xfl8", name="xfl8")
                for c in range(NDC):
                    nc.sync.dma_start(xfl8_sb[:, c, :], xfl8[:, c * SL:(c + 1) * SL])

            if USE_FP8_SCORES:
                R8_sb = P.tile([TS, NDC, S], FP8, tag="R8", name="R8")
            else:
                R_sb = [P.tile([TS, S], BF16, tag=f"R{dc}", name=f"R{dc}") for dc in range(NDC)]
            Vt_sb = [P.tile([TS, D], BF16, tag=f"Vt{kt}", name=f"Vt{kt}") for kt in range(NLT)]
            # ones column for the l row-sum matmul (l = ones^T @ P^T, M=1)
            ones_sb = P.tile([TS, 1], BF16, tag="ones", name="ones")
            nc.vector.memset(ones_sb[:], 1.0)
            # per-qb row sums land here; one lrow DMA at the end
            lall_sb = P.tile([1, S], F32, tag="lall", name="lall")

            from contextlib import ExitStack
            with ExitStack() as rep_ctx:
                if repeat > 1:
                    rep_ctx.enter_context(tc.For_i(0, repeat, 1))
                # ---- projections ----
                with tc.tile_pool(name="proj_ps", bufs=4, space="PSUM") as PP:
                    # Vt[k,e]: lhsT = xfl[c, k-chunk], rhs = H[c, :]
                    for kt in range(NLT):
                        ps = PP.tile([TS, D], F32, tag="ps", name="ps_v")
                        for e in range(NDC):
                            nc.tensor.matmul(
                                ps[:], xfl_sb[e][:, kt * TS:(kt + 1) * TS], h_sb[e][:],
                                start=(e == 0), stop=(e == NDC - 1))
                        if kt % 2 == 0:
                            nc.vector.tensor_copy(Vt_sb[kt][:], ps[:])
                        else:
                            nc.scalar.activation(Vt_sb[kt][:], ps[:],
                                                 mybir.ActivationFunctionType.Copy)
                # R[b,q] = (xG)^T: lhsT = G[a, b-chunk], rhs = xT[a, colblock].
                # 8 parallel accumulation chains (one PSUM bank per colblock)
                # so each stationary G chunk is loaded once and streams 8
                # colblocks back-to-back.
                with tc.tile_pool(name="rproj_ps", bufs=1, space="PSUM") as RP:
                    for dc in range(NDC):
                        pss = [RP.tile([TS, QB], F32, tag=f"ps{cb}", name=f"ps_p{cb}")
                               for cb in range(S // QB)]
                        if USE_FP8_SCORES:
                            for g2 in range(2):
                                for cb in range(S // QB):
                                    nc.tensor.matmul(
                                        pss[cb][:],
                                        g8_sb[:, 2 * g2:2 * g2 + 2, dc * TS:(dc + 1) * TS],
                                        xT8_sb[:, 2 * g2:2 * g2 + 2, cb * QB:(cb + 1) * QB],
                                        start=(g2 == 0), stop=(g2 == 1),
                                        perf_mode=mybir.MatmulPerfMode.DoubleRow)
                        else:
                            for e in range(NDC):
                                for cb in range(S // QB):
                                    nc.tensor.matmul(
                                        pss[cb][:], g_sb[e][:, dc * TS:(dc + 1) * TS],
                                        xT_sb[e][:, cb * QB:(cb + 1) * QB],
                                        start=(e == 0), stop=(e == NDC - 1))
                        for cb in range(S // QB):
                            if USE_FP8_SCORES:
                                dst = R8_sb[:, dc, cb * QB:(cb + 1) * QB]
                            else:
                                dst = R_sb[dc][:, cb * QB:(cb + 1) * QB]
                            # split PSUM drains across DVE and ScalarE: the
                            # R-proj phase is copy-bound on a single engine
                            if cb % 2 == 0:
                                nc.vector.tensor_copy(dst, pss[cb][:])
                            else:
                                nc.scalar.activation(dst, pss[cb][:],
                                                     mybir.ActivationFunctionType.Copy)

                # ---- attention (directly in output space), per query block ----
                # Software-pipelined attention: the PE program interleaves the
                # score matmuls of step s+2 ahead of the exp-dependent l/PV
                # matmuls of step s, so the in-order PE never waits on ScalarE.
                with tc.tile_pool(name="st_ps", bufs=3, space="PSUM") as STP, \
                     tc.tile_pool(name="attn_ps", bufs=1, space="PSUM") as ATP, \
                     tc.tile_pool(name="l_ps", bufs=1, space="PSUM") as LP, \
                     tc.tile_pool(name="p_sb", bufs=8) as PSB, \
                     tc.tile_pool(name="o_sb", bufs=3) as OSB:
                    steps = [(j, lt) for j in range(NQB) for lt in range(2 * j + 2)]
                    nsteps = len(steps)
                    p_t = {}
                    attn_ps = None
                    l_ps = None

                    def issue_score(s):
                        j, lt = steps[s]
                        qcol = slice(j * QB, (j + 1) * QB)
                        st = STP.tile([TS, QB], F32, tag="st", name="st")
                        if USE_FP8_SCORES:
                            for gg in range(2):
                                nc.tensor.matmul(
                                    st[:],
                                    xfl8_sb[:, 2 * gg:2 * gg + 2, lt * TS:(lt + 1) * TS],
                                    R8_sb[:, 2 * gg:2 * gg + 2, qcol],
                                    start=(gg == 0), stop=(gg == 1),
                                    perf_mode=mybir.MatmulPerfMode.DoubleRow)
                        else:
                            for dc in range(NDC):
                                nc.tensor.matmul(
                                    st[:], xfl_sb[dc][:, lt * TS:(lt + 1) * TS], R_sb[dc][:, qcol],
                                    start=(dc == 0), stop=(dc == NDC - 1))
                        p = PSB.tile([TS, QB], BF16, tag="p", name="p")
                        nc.scalar.activation(
                            p[:], st[:], mybir.ActivationFunctionType.Exp, scale=SCALE)
                        if lt >= 2 * j:
                            nc.vector.tensor_mul(p[:], p[:], mask_sb[lt - 2 * j][:])
                        p_t[s] = p

                    def issue_pv(s):
                        nonlocal attn_ps, l_ps
                        j, lt = steps[s]
                        nlt = 2 * j + 2
                        qcol = slice(j * QB, (j + 1) * QB)
                        if lt == 0:
                            attn_ps = [ATP.tile([TS, QB], F32, tag=f"attn{dc}", name=f"attn{dc}")
                                       for dc in range(NDC)]
                            l_ps = LP.tile([1, QB], F32, tag="l_ps", name="l_ps")
                        p = p_t.pop(s)
                        # l row-sum via M=1 ones-matmul: accumulates in PSUM
                        # alongside attn, no serial engine chain.
                        nc.tensor.matmul(
                            l_ps[:], ones_sb[:], p[:],
                            start=(lt == 0), stop=(lt == nlt - 1))
                        for dc in range(NDC):
                            nc.tensor.matmul(
                                attn_ps[dc][:], Vt_sb[lt][:, dc * TS:(dc + 1) * TS], p[:],
                                start=(lt == 0), stop=(lt == nlt - 1))
                        if lt == nlt - 1:
                            nc.scalar.activation(lall_sb[0:1, qcol], l_ps[0:1, :],
                                                 mybir.ActivationFunctionType.Copy)
                            for dc in range(NDC):
                                po_sb = OSB.tile([TS, QB], BF16, tag=f"po_sb{dc}", name=f"po_sb{dc}")
                                for half in range(2):
                                    hs = slice(half * (QB // 2), (half + 1) * (QB // 2))
                                    # alternate engines to release PSUM banks sooner
                                    eng = nc.vector if (dc + half) % 2 == 0 else nc.scalar
                                    if eng is nc.scalar:
                                        nc.scalar.activation(po_sb[:, hs], attn_ps[dc][:, hs],
                                                             mybir.ActivationFunctionType.Copy)
                                    else:
                                        nc.vector.tensor_copy(po_sb[:, hs], attn_ps[dc][:, hs])
                                nc.sync.dma_start(poT[dc * TS:(dc + 1) * TS, qcol], po_sb[:])

                    issue_score(0)
                    issue_score(1)
                    for s in range(nsteps):
                        if s + 2 < nsteps:
                            issue_score(s + 2)
                        issue_pv(s)
                    nc.sync.dma_start(lrow[0:1, :], lall_sb[0:1, :])
    nc.compile()
    return nc


_cache = {}


def _make_masks(h):
    m = np.zeros((2 * TS, QB), dtype=np.float32)
    k_r = np.arange(TS)[:, None]
    q_r = np.arange(QB)[None, :]
    for c in range(2):
        m[c * TS:(c + 1) * TS] = (q_r >= 128 * (2 * c + h) + k_r)
    return m.astype(ml_dtypes.bfloat16)


def kernel(x, Wq, Wk, Wv, Wo, bo):
    bf = ml_dtypes.bfloat16
    x = np.asarray(x, dtype=np.float32)
    Wq, Wk, Wv, Wo, bo = (np.asarray(a, dtype=np.float32) for a in (Wq, Wk, Wv, Wo, bo))
    if "nc" not in _cache:
        _cache["nc"] = build_kernel()
    nc = _cache["nc"]

    # fold the projections: scores = x G x^T, Vt = x H (= V Wo^T)
    Gf = np.ascontiguousarray(Wq.T @ Wk)
    G = Gf.astype(bf)
    H = np.ascontiguousarray(Wv.T @ Wo.T).astype(bf)
    mask_h = [_make_masks(0), _make_masks(1)]

    # local kv columns for parity h: 128-col tiles with global tile index % 2 == h
    col_idx = {}
    for h in range(2):
        tiles = [np.arange(TS * (2 * lt + h), TS * (2 * lt + h) + TS) for lt in range(NLT)]
        col_idx[h] = np.concatenate(tiles)

    in_maps = []
    for core in range(N_CORES):
        b, h = core // 2, core % 2
        xTb = np.ascontiguousarray(x[b].T).astype(bf)     # [D, S]
        xflb = np.ascontiguousarray(xTb[:, col_idx[h]])
        im = {
            "xfl": xflb,
            "h": H,
            "masks": mask_h[h],
        }
        if USE_FP8_SCORES:
            f8 = ml_dtypes.float8_e4m3
            xfl_f32 = x[b].T[:, col_idx[h]].astype(np.float32)
            im["xfl8"] = np.ascontiguousarray(
                xfl_f32.reshape(NDC, TS, SL).transpose(1, 0, 2).reshape(TS, NDC * SL)
            ).astype(f8)
            im["xT8"] = np.ascontiguousarray(
                x[b].T.astype(np.float32).reshape(NDC, TS, S).transpose(1, 0, 2).reshape(TS, NDC * S)
            ).astype(f8)
            im["g8"] = np.ascontiguousarray(
                Gf.reshape(NDC, TS, D).transpose(1, 0, 2).reshape(TS, NDC * D)
            ).astype(f8)
        else:
            im["xT"] = xTb
            im["g"] = G
        in_maps.append(im)

    global _last_in_maps
    _last_in_maps = in_maps
    res = bass_utils.run_bass_kernel_spmd(nc, in_maps, core_ids=list(range(N_CORES)))

    out = np.zeros((B, S, D), dtype=np.float32)
    for b in range(B):
        r0, r1 = res.results[2 * b], res.results[2 * b + 1]
        l = (r0["lrow"] + r1["lrow"]).reshape(1, S)
        poTs = r0["poT"].astype(np.float32) + r1["poT"].astype(np.float32)
        out[b] = (poTs / l).T + bo.astype(np.float32)
    return out

